# revision 5
# baseline (speedup 1.0000x reference)
"""Trainium2 Bass kernel for nn_Net_89094801588965 (moe_routing).

Data-parallel over batch on 8 NeuronCores. Per-core layout puts features on
SBUF partitions and batch on the free dim, so every layer's output is directly
the next layer's moving operand (no transposes on device).

Math (identical to the reference):
  h  = relu(x @ fc1_w + b) -> relu(@fc2_w+b) -> relu(@fc3_w+b)   [B,256]
  p  = relu(x @ priv_w[task_id] + priv_b[task_id])               [B,256]
  xc = [p, h]                                                    [B,512]
  per-task heads t=0..9: a3[t] = (relu(relu(xc@h1w[t]+b)@h2w[t]+b))@h3w[t]+b
  out[b] = a3[tt[b]][b]

Device-side restructuring:
  - fc1 and the private layer share the input x -> fused into one [784,656]
    matmul (cols 0..255 = private, 256..655 = fc1).
  - head layer 1: all tasks packed as [512, 320] (task t at cols 32t..32t+27,
    zero padded) -> [320, N] activations.
  - head layer 2: block-diagonal [320, 320], 128-aligned diagonal blocks ->
    3 matmuls (tasks 0-3, 4-7, 8-9).
  - head layer 3: block-structured [320, 100] -> a3 for all tasks as [100, N].
  - selection: one-hot mask over tasks, expanded to 100 rows on the host,
    applied as (a3 + bias) * mask, then reduced with a constant [100, 10]
    summing matrix on the PE.  (relu/bias of wrong tasks is killed by the
    mask, so masking once at the end is exact.)
All matmuls run as float32r (full PE rate, ~1e-4 relative error).
"""

import sys

sys.path.insert(0, "/opt/trn_rl_repo")

import numpy as np

import concourse.bass as bass
import concourse.mybir as mybir
import concourse.tile as tile
from concourse import bacc
from concourse.bass_utils import run_bass_kernel_spmd

F32 = mybir.dt.float32
F32R = mybir.dt.float32r
RELU = mybir.ActivationFunctionType.Relu

B = 65536
D = 784
HID = 400
LAT = 256
T = 10
NCLS = 10
NCORES = 8
R = B // NCORES          # rows per core
CH = 512                 # batch columns per chunk
NCH = R // CH            # chunks per core

M1 = LAT + HID           # 656 fused L1 output (private | fc1)
HP = 32                  # per-task padded head width
HT = T * HP              # 320
A3 = T * NCLS            # 100
MSK = HT + T             # 330: [mask320 ; mask10] rows
H3_KM = [128, 128, HT + T - 256]   # k-tiles of the masked head-3 contraction

_cache = {}


def _ceil_tiles(n):
    full, rem = divmod(n, 128)
    sizes = [128] * full
    if rem:
        sizes.append(rem)
    return sizes


L1_K = _ceil_tiles(D)            # [128]*6 + [16]
L1_M = _ceil_tiles(M1)           # [128]*5 + [16]
L2_K = _ceil_tiles(HID)          # [128]*3 + [16]
L2_M = _ceil_tiles(HID)
L3_M = _ceil_tiles(LAT)          # [128, 128]
H1_K = _ceil_tiles(2 * LAT)      # [128]*4
H1_M = _ceil_tiles(HT)           # [128, 128, 64]
H3_K = _ceil_tiles(HT)           # [128, 128, 64]


def _build_program():
    nc = bacc.Bacc("TRN2", target_bir_lowering=False, debug=False,
                   num_devices=NCORES)

    xT_d = nc.dram_tensor("xT", [D, R], F32R, kind="ExternalInput")
    mk_d = nc.dram_tensor("mask", [MSK, R], F32, kind="ExternalInput")
    w1_d = nc.dram_tensor("w1", [D, M1], F32R, kind="ExternalInput")
    w2_d = nc.dram_tensor("w2", [HID, HID], F32R, kind="ExternalInput")
    w3_d = nc.dram_tensor("w3", [HID, LAT], F32R, kind="ExternalInput")
    wh1_d = nc.dram_tensor("wh1", [2 * LAT, HT], F32R, kind="ExternalInput")
    wh2_d = nc.dram_tensor("wh2", [HT, HT], F32R, kind="ExternalInput")
    wh3_d = nc.dram_tensor("wh3", [MSK, NCLS], F32R, kind="ExternalInput")
    b1_d = nc.dram_tensor("b1", [128, len(L1_M)], F32, kind="ExternalInput")
    b2_d = nc.dram_tensor("b2", [128, len(L2_M)], F32, kind="ExternalInput")
    b3_d = nc.dram_tensor("b3", [128, len(L3_M)], F32, kind="ExternalInput")
    bh1_d = nc.dram_tensor("bh1", [128, len(H1_M)], F32, kind="ExternalInput")
    bh2_d = nc.dram_tensor("bh2", [128, len(H1_M)], F32, kind="ExternalInput")
    out_d = nc.dram_tensor("out", [NCLS, R], F32, kind="ExternalOutput")

    with tile.TileContext(nc) as tc:
        with (
            tc.tile_pool(name="wp", bufs=1) as wp,
            tc.tile_pool(name="xp", bufs=14) as xp,
            tc.tile_pool(name="mp", bufs=2) as mpool,
            tc.tile_pool(name="ap", bufs=2) as ap,
            tc.tile_pool(name="op", bufs=2) as op,
            tc.tile_pool(name="ps", bufs=8, space="PSUM") as ps,
        ):
            # ---- resident weights -------------------------------------
            def load_w(dram, ksizes, ncols, tag):
                tiles = []
                r0 = 0
                for i, kp in enumerate(ksizes):
                    t = wp.tile([kp, ncols], F32R, tag=f"{tag}{i}")
                    nc.sync.dma_start(t[:], dram[r0:r0 + kp, :])
                    tiles.append(t)
                    r0 += kp
                return tiles

            def load_x_chunk(ci):
                cs = ci * CH
                xk = []
                r0 = 0
                for ki, kp in enumerate(L1_K):
                    t = xp.tile([kp, CH], F32R, tag="x")
                    nc.sync.dma_start(t[:], xT_d[r0:r0 + kp, cs:cs + CH])
                    xk.append(t)
                    r0 += kp
                return xk

            # interleave W1 and chunk-0 x so the first matmul starts early
            w1, x0k = [], []
            r0 = 0
            for i, kp in enumerate(L1_K):
                wt = wp.tile([kp, M1], F32R, tag=f"w1{i}")
                nc.sync.dma_start(wt[:], w1_d[r0:r0 + kp, :])
                w1.append(wt)
                xt = xp.tile([kp, CH], F32R, tag="x")
                nc.sync.dma_start(xt[:], xT_d[r0:r0 + kp, 0:CH])
                x0k.append(xt)
                r0 += kp
            x1k = load_x_chunk(1)
            w2 = load_w(w2_d, L2_K, HID, "w2")
            w3 = load_w(w3_d, L2_K, LAT, "w3")
            wh1 = load_w(wh1_d, H1_K, HT, "wh1")
            wh3 = load_w(wh3_d, H3_KM, NCLS, "wh3")
            # block-diagonal head-2: only the diagonal 128-blocks
            wh2 = []
            r0 = 0
            for i, kp in enumerate(H3_K):
                t = wp.tile([kp, kp], F32R, tag=f"wh2{i}")
                nc.sync.dma_start(t[:], wh2_d[r0:r0 + kp, r0:r0 + kp])
                wh2.append(t)
                r0 += kp

            def load_b(dram, ncols, tag):
                t = wp.tile([128, ncols], F32, tag=tag)
                nc.sync.dma_start(t[:], dram[:])
                return t

            b1 = load_b(b1_d, len(L1_M), "b1")
            b2 = load_b(b2_d, len(L2_M), "b2")
            b3 = load_b(b3_d, len(L3_M), "b3")
            bh1 = load_b(bh1_d, len(H1_M), "bh1")
            bh2 = load_b(bh2_d, len(H1_M), "bh2")

            # ---- per-chunk pipeline -----------------------------------
            def mm_layer(ktiles, wtiles, msizes, psum_tag_prefix, ci):
                """K-accumulated matmuls for one dense layer; returns psum
                tiles (one per m-tile)."""
                psums = []
                c0 = 0
                for mi, mp_ in enumerate(msizes):
                    pt = ps.tile([mp_, CH], F32, tag="ps")
                    nk = len(ktiles)
                    for ki in range(nk):
                        nc.tensor.matmul(
                            pt[:], wtiles[ki][:, c0:c0 + mp_],
                            ktiles[ki][:],
                            start=(ki == 0), stop=(ki == nk - 1),
                        )
                    psums.append(pt)
                    c0 += mp_
                return psums

            def act_relu(psums, bias, msizes, tag, ci, eng="act"):
                outs = []
                for mi, mp_ in enumerate(msizes):
                    t = ap.tile([mp_, CH], F32R, tag=f"{tag}{mi}")
                    if eng == "act":
                        nc.scalar.activation(t[:], psums[mi][:], RELU,
                                             bias=bias[:mp_, mi:mi + 1],
                                             scale=1.0)
                    else:
                        nc.vector.tensor_scalar(
                            t[:], psums[mi][:], bias[:mp_, mi:mi + 1], 0.0,
                            op0=mybir.AluOpType.add, op1=mybir.AluOpType.max)
                    outs.append(t)
                return outs

            deferred_tail = []

            for ci in range(NCH):
                cs = ci * CH
                xk = x0k if ci == 0 else (x1k if ci == 1 else load_x_chunk(ci))
                # mask tiles: [0:128), [128:256), [256:330) rows of mask330
                mks = []
                r0 = 0
                for mi, kp in enumerate(H3_KM):
                    t = mpool.tile([kp, CH], F32, tag=f"mask{mi}")
                    nc.sync.dma_start(t[:], mk_d[r0:r0 + kp, cs:cs + CH])
                    mks.append(t)
                    r0 += kp

                # L1 fused (private | fc1)
                ps1 = mm_layer(xk, w1, L1_M, "l1", ci)
                a_l1 = act_relu(ps1, b1, L1_M, "l1o", ci)
                x2 = [a_l1[0], a_l1[1]]
                h1t = [a_l1[2], a_l1[3], a_l1[4], a_l1[5]]

                # previous chunk's tail overlaps this chunk's L1 on PE
                while deferred_tail:
                    deferred_tail.pop(0)()

                ps2 = mm_layer(h1t, w2, L2_M, "l2", ci)
                h2t = act_relu(ps2, b2, L2_M, "l2o", ci, eng="dve")
                ps3 = mm_layer(h2t, w3, L3_M, "l3", ci)
                x2 += act_relu(ps3, b3, L3_M, "l3o", ci, eng="dve")

                ph1 = mm_layer(x2, wh1, H1_M, "h1", ci)
                a1 = act_relu(ph1, bh1, H1_M, "a1", ci)

                def tail(a1=a1, mks=mks, cs=cs):
                    # head layer 2: block diagonal, 3 independent matmuls
                    ph2 = []
                    for i, kp in enumerate(H3_K):
                        pt = ps.tile([kp, CH], F32, tag="ps")
                        nc.tensor.matmul(pt[:], wh2[i][:], a1[i][:],
                                         start=True, stop=True)
                        ph2.append(pt)
                    a2 = act_relu(ph2, bh2, H1_M, "a2", ci, eng="dve")
                    # mask the per-task activations; k-tile 2 also carries the
                    # one-hot rows that select the per-task bias h3_b
                    am = []
                    for i in range(2):
                        t = ap.tile([128, CH], F32R, tag=f"am{i}")
                        nc.vector.tensor_tensor(t[:], a2[i][:], mks[i][:],
                                                op=mybir.AluOpType.mult)
                        am.append(t)
                    t2 = ap.tile([H3_KM[2], CH], F32R, tag="am2")
                    nc.vector.tensor_tensor(t2[0:64, :], a2[2][:], mks[2][0:64, :],
                                            op=mybir.AluOpType.mult)
                    nc.vector.tensor_copy(t2[64:, :], mks[2][64:, :])
                    am.append(t2)
                    # head layer 3 + task-select + bias in one contraction
                    po = ps.tile([NCLS, CH], F32, tag="ps")
                    for i in range(3):
                        nc.tensor.matmul(po[:], wh3[i][:], am[i][:],
                                         start=(i == 0), stop=(i == 2))
                    ot = op.tile([NCLS, CH], F32, tag="o")
                    nc.scalar.copy(ot[:], po[:])
                    nc.sync.dma_start(out_d[:, cs:cs + CH], ot[:])

                deferred_tail.append(tail)

            while deferred_tail:
                deferred_tail.pop(0)()

    nc.compile()
    return nc


def _prepare_inputs(x_s, tt, task_id,
                    fc1_w, fc1_b, fc2_w, fc2_b, fc3_w, fc3_b,
                    priv_w, priv_b, h1_w, h1_b, h2_w, h2_b, h3_w, h3_b):
    f = np.float32
    task_id = int(task_id)

    x2d = np.asarray(x_s, f).reshape(B, D)
    tt = np.asarray(tt).astype(np.int64).reshape(B)

    w1 = np.concatenate([np.asarray(priv_w[task_id], f),
                         np.asarray(fc1_w, f)], axis=1)          # [784, 656]
    b1v = np.concatenate([np.asarray(priv_b[task_id], f),
                          np.asarray(fc1_b, f)])                  # [656]
    w2 = np.ascontiguousarray(np.asarray(fc2_w, f))
    w3 = np.ascontiguousarray(np.asarray(fc3_w, f))
    b2v = np.asarray(fc2_b, f)
    b3v = np.asarray(fc3_b, f)

    wh1 = np.zeros((2 * LAT, HT), f)
    bh1v = np.zeros(HT, f)
    wh2 = np.zeros((HT, HT), f)
    bh2v = np.zeros(HT, f)
    wh3 = np.zeros((MSK, NCLS), f)
    for t in range(T):
        c = HP * t
        wh1[:, c:c + 28] = np.asarray(h1_w[t], f)
        bh1v[c:c + 28] = np.asarray(h1_b[t], f)
        wh2[c:c + 28, c:c + 28] = np.asarray(h2_w[t], f)
        bh2v[c:c + 28] = np.asarray(h2_b[t], f)
        wh3[c:c + 28, :] = np.asarray(h3_w[t], f)
        wh3[HT + t, :] = np.asarray(h3_b[t], f)

    def col_bias(v, msizes):
        out = np.zeros((128, len(msizes)), f)
        r0 = 0
        for i, mp_ in enumerate(msizes):
            out[:mp_, i] = v[r0:r0 + mp_]
            r0 += mp_
        return out

    shared = {
        "w1": w1, "w2": w2, "w3": w3, "wh1": wh1, "wh2": wh2, "wh3": wh3,
        "b1": col_bias(b1v, L1_M), "b2": col_bias(b2v, L2_M),
        "b3": col_bias(b3v, L3_M), "bh1": col_bias(bh1v, H1_M),
        "bh2": col_bias(bh2v, H1_M),
    }

    in_maps = []
    for c in range(NCORES):
        sl = slice(c * R, (c + 1) * R)
        xT = np.ascontiguousarray(x2d[sl].T)                     # [784, R]
        ttc = tt[sl]
        m10 = (np.arange(T)[:, None] == ttc[None, :])
        mask = np.concatenate([np.repeat(m10, HP, axis=0), m10],
                              axis=0).astype(f)                  # [330, R]
        m = dict(shared)
        m["xT"] = xT
        m["mask"] = mask
        in_maps.append(m)
    return in_maps


def run(inputs, trace=False, **kw):
    if "nc" not in _cache:
        _cache["nc"] = _build_program()
    nc = _cache["nc"]
    inputs = {k: v for k, v in inputs.items() if k != "x_p"}
    in_maps = _prepare_inputs(**inputs)
    res = run_bass_kernel_spmd(nc, in_maps, list(range(NCORES)),
                               trace=trace, **kw)
    outs = [res.results[c]["out"] for c in range(NCORES)]        # [10, R] each
    full = np.concatenate(outs, axis=1)                          # [10, B]
    return np.ascontiguousarray(full.T), res                     # [B, 10]


def kernel(**inputs):
    out, _ = run(inputs, trace=False)
    return out


# revision 6
# speedup vs baseline: 1.0679x; 1.0679x over previous
"""Trainium2 Bass kernel for nn_Net_89094801588965 (moe_routing).

Data-parallel over batch on 8 NeuronCores. Per-core layout puts features on
SBUF partitions and batch on the free dim, so every layer's output is directly
the next layer's moving operand (no transposes on device).

Math (identical to the reference):
  h  = relu(x @ fc1_w + b) -> relu(@fc2_w+b) -> relu(@fc3_w+b)   [B,256]
  p  = relu(x @ priv_w[task_id] + priv_b[task_id])               [B,256]
  xc = [p, h]                                                    [B,512]
  per-task heads t=0..9: a3[t] = (relu(relu(xc@h1w[t]+b)@h2w[t]+b))@h3w[t]+b
  out[b] = a3[tt[b]][b]

Device-side restructuring:
  - fc1 and the private layer share the input x -> fused into one [784,656]
    matmul (cols 0..255 = private, 256..655 = fc1).
  - head layer 1: all tasks packed as [512, 320] (task t at cols 32t..32t+27,
    zero padded) -> [320, N] activations.
  - head layer 2: block-diagonal [320, 320], 128-aligned diagonal blocks ->
    3 matmuls (tasks 0-3, 4-7, 8-9).
  - head layer 3 + routing: multiply a2 by the per-task one-hot mask (built on
    the host from tt), then contract with the stacked [330, 10] weight whose
    rows 320..329 are h3_b paired with raw one-hot mask rows -- a single
    contraction yields the routed, biased logits.  Masking only at the end is
    exact because relu/bias garbage of non-selected tasks is zeroed there.
  - all big tensors are host-padded to 128-row multiples and loaded with ONE
    dma_start each (descriptor generation is serialized at ~0.6us per DMA).
  - per-chunk head-2/head-3 work is software-pipelined into the next chunk's
    L1/L2/L3 shadow so the PE never waits on ACT/DVE round-trips.
All matmuls run as float32r (full PE rate, ~1e-4 relative error).
"""

import sys

sys.path.insert(0, "/opt/trn_rl_repo")

import numpy as np

import concourse.bass as bass
import concourse.mybir as mybir
import concourse.tile as tile
from concourse import bacc
from concourse.bass_utils import run_bass_kernel_spmd

F32 = mybir.dt.float32
F32R = mybir.dt.float32r
RELU = mybir.ActivationFunctionType.Relu

B = 65536
D = 784
HID = 400
LAT = 256
T = 10
NCLS = 10
NCORES = 8
R = B // NCORES          # rows per core
CH = 512                 # batch columns per chunk
NCH = R // CH            # chunks per core

M1 = LAT + HID           # 656 fused L1 output (private | fc1)
HP = 32                  # per-task padded head width
HT = T * HP              # 320
MSK = HT + T             # 330 mask rows: [expanded one-hot ; raw one-hot]

_cache = {}


def _ceil_tiles(n):
    full, rem = divmod(n, 128)
    return [128] * full + ([rem] if rem else [])


L1_K = _ceil_tiles(D)            # [128]*6 + [16]
L1_M = _ceil_tiles(M1)           # [128]*5 + [16]
L2_K = _ceil_tiles(HID)          # [128]*3 + [16]
L2_M = _ceil_tiles(HID)
L3_M = _ceil_tiles(LAT)          # [128, 128]
H1_K = _ceil_tiles(2 * LAT)      # [128]*4
H1_M = _ceil_tiles(HT)           # [128, 128, 64]
H3_K = _ceil_tiles(MSK)          # [128, 128, 74]

# bias column layout inside the single [128, 18] bias tensor
BC_L1, BC_L2, BC_L3, BC_H1, BC_H2 = 0, 6, 10, 12, 15
NBC = 18


def _pad128(n):
    return 128 * ((n + 127) // 128)


def _build_program():
    nc = bacc.Bacc("TRN2", target_bir_lowering=False, debug=False,
                   num_devices=NCORES)

    xT_d = nc.dram_tensor("xT", [_pad128(D), R], F32R, kind="ExternalInput")
    mk_d = nc.dram_tensor("mask", [_pad128(MSK), R], F32, kind="ExternalInput")
    w1_d = nc.dram_tensor("w1", [_pad128(D), M1], F32R, kind="ExternalInput")
    w2_d = nc.dram_tensor("w2", [_pad128(HID), HID], F32R, kind="ExternalInput")
    w3_d = nc.dram_tensor("w3", [_pad128(HID), LAT], F32R, kind="ExternalInput")
    wh1_d = nc.dram_tensor("wh1", [2 * LAT, HT], F32R, kind="ExternalInput")
    wh2_d = nc.dram_tensor("wh2", [3 * 128, 128], F32R, kind="ExternalInput")
    wh3_d = nc.dram_tensor("wh3", [_pad128(MSK), NCLS], F32R,
                           kind="ExternalInput")
    bias_d = nc.dram_tensor("bias", [128, NBC], F32, kind="ExternalInput")
    out_d = nc.dram_tensor("out", [NCLS, R], F32, kind="ExternalOutput")

    def as3d(dram):
        return dram[:].rearrange("(j p) m -> p j m", p=128)

    with tile.TileContext(nc) as tc:
        with (
            tc.tile_pool(name="wp", bufs=1) as wp,
            tc.tile_pool(name="xp", bufs=2) as xp,
            tc.tile_pool(name="mp", bufs=3) as mpool,
            tc.tile_pool(name="ap", bufs=2) as ap,
            tc.tile_pool(name="op", bufs=2) as op,
            tc.tile_pool(name="ps", bufs=8, space="PSUM") as ps,
        ):
            # ---- resident weights: one DMA per tensor --------------------
            def load_w3d(dram, nk, ncols, tag):
                t = wp.tile([128, nk, ncols], F32R, tag=tag)
                nc.sync.dma_start(t[:], as3d(dram))
                return t

            def load_x_chunk(ci):
                t = xp.tile([128, len(L1_K), CH], F32R, tag="x")
                nc.sync.dma_start(
                    t[:], as3d(xT_d)[:, :, ci * CH:(ci + 1) * CH])
                return t

            w1 = load_w3d(w1_d, len(L1_K), M1, "w1")
            x0 = load_x_chunk(0)
            x1 = load_x_chunk(1)
            w2 = load_w3d(w2_d, len(L2_K), HID, "w2")
            w3 = load_w3d(w3_d, len(L2_K), LAT, "w3")
            wh1 = load_w3d(wh1_d, len(H1_K), HT, "wh1")
            wh2 = load_w3d(wh2_d, 3, 128, "wh2")
            wh3 = load_w3d(wh3_d, len(H3_K), NCLS, "wh3")
            bias = wp.tile([128, NBC], F32, tag="bias")
            nc.sync.dma_start(bias[:], bias_d[:])

            # ---- helpers -------------------------------------------------
            def mm_layer(rhs3, ksizes, w3t, msizes, rhs_list=None):
                """K-accumulated matmuls; rhs3 is a [128, nk, CH] tile or
                rhs_list a list of [kp, CH] tiles."""
                psums = []
                c0 = 0
                for mi, mp_ in enumerate(msizes):
                    pt = ps.tile([mp_, CH], F32, tag="ps")
                    nk = len(ksizes)
                    for ki, kp in enumerate(ksizes):
                        rhs = (rhs3[0:kp, ki, :] if rhs3 is not None
                               else rhs_list[ki][:])
                        nc.tensor.matmul(
                            pt[:], w3t[0:kp, ki, c0:c0 + mp_], rhs,
                            start=(ki == 0), stop=(ki == nk - 1),
                        )
                    psums.append(pt)
                    c0 += mp_
                return psums

            def act_relu(psums, bcol, msizes, tag, eng="act"):
                outs = []
                for mi, mp_ in enumerate(msizes):
                    t = ap.tile([mp_, CH], F32R, tag=f"{tag}{mi}")
                    bap = bias[:mp_, bcol + mi:bcol + mi + 1]
                    if eng == "act":
                        nc.scalar.activation(t[:], psums[mi][:], RELU,
                                             bias=bap, scale=1.0)
                    else:
                        nc.vector.tensor_scalar(
                            t[:], psums[mi][:], bap, 0.0,
                            op0=mybir.AluOpType.add, op1=mybir.AluOpType.max)
                    outs.append(t)
                return outs

            tails_h2 = []
            tails_h3 = []

            for ci in range(NCH):
                cs = ci * CH
                xk = x0 if ci == 0 else (x1 if ci == 1 else load_x_chunk(ci))
                mk = mpool.tile([128, len(H3_K), CH], F32, tag="mask")
                nc.sync.dma_start(mk[:], as3d(mk_d)[:, :, cs:cs + CH])

                # L1 fused (private | fc1)
                ps1 = mm_layer(xk, L1_K, w1, L1_M)
                a_l1 = act_relu(ps1, BC_L1, L1_M, "l1o")
                x2 = [a_l1[0], a_l1[1]]
                h1t = [a_l1[2], a_l1[3], a_l1[4], a_l1[5]]

                # previous chunk's head-2 runs inside this chunk's L1 shadow
                while tails_h2:
                    tails_h2.pop(0)()

                ps2 = mm_layer(None, L2_K, w2, L2_M, rhs_list=h1t)
                h2t = act_relu(ps2, BC_L2, L2_M, "l2o", eng="dve")
                ps3 = mm_layer(None, L2_K, w3, L3_M, rhs_list=h2t)
                x2 += act_relu(ps3, BC_L3, L3_M, "l3o", eng="dve")

                # previous chunk's head-3 + store
                while tails_h3:
                    tails_h3.pop(0)()

                ph1 = mm_layer(None, H1_K, wh1, H1_M, rhs_list=x2)
                a1 = act_relu(ph1, BC_H1, H1_M, "a1")

                def tail_h2(a1=a1, mk=mk):
                    ph2 = []
                    for i, kp in enumerate(H1_M):
                        pt = ps.tile([kp, CH], F32, tag="ps")
                        nc.tensor.matmul(pt[:], wh2[0:kp, i, 0:kp], a1[i][:],
                                         start=True, stop=True)
                        ph2.append(pt)
                    a2 = act_relu(ph2, BC_H2, H1_M, "a2")
                    # mask the per-task activations; the 74-row tile also
                    # carries the raw one-hot rows that select h3_b
                    am = []
                    for i in range(2):
                        t = ap.tile([128, CH], F32R, tag=f"am{i}")
                        nc.vector.tensor_tensor(t[:], a2[i][:], mk[:, i, :],
                                                op=mybir.AluOpType.mult)
                        am.append(t)
                    t2 = ap.tile([H3_K[2], CH], F32R, tag="am2")
                    nc.vector.tensor_tensor(t2[0:64, :], a2[2][:],
                                            mk[0:64, 2, :],
                                            op=mybir.AluOpType.mult)
                    nc.vector.tensor_copy(t2[64:, :], mk[64:H3_K[2], 2, :])
                    am.append(t2)
                    return am

                def tail_h3(am, cs=cs):
                    po = ps.tile([NCLS, CH], F32, tag="ps")
                    for i, kp in enumerate(H3_K):
                        nc.tensor.matmul(po[:], wh3[0:kp, i, :], am[i][:],
                                         start=(i == 0), stop=(i == 2))
                    ot = op.tile([NCLS, CH], F32, tag="o")
                    nc.scalar.copy(ot[:], po[:])
                    nc.sync.dma_start(out_d[:, cs:cs + CH], ot[:])

                def chain(t2=tail_h2, t3=tail_h3):
                    am = t2()
                    tails_h3.append(lambda: t3(am))

                tails_h2.append(chain)

            while tails_h2:
                tails_h2.pop(0)()
            while tails_h3:
                tails_h3.pop(0)()

    nc.compile()
    return nc


def _prepare_inputs(x_s, tt, task_id,
                    fc1_w, fc1_b, fc2_w, fc2_b, fc3_w, fc3_b,
                    priv_w, priv_b, h1_w, h1_b, h2_w, h2_b, h3_w, h3_b):
    f = np.float32
    task_id = int(task_id)

    x2d = np.asarray(x_s, f).reshape(B, D)
    tt = np.asarray(tt).astype(np.int64).reshape(B)

    w1 = np.zeros((_pad128(D), M1), f)
    w1[:D, :LAT] = np.asarray(priv_w[task_id], f)
    w1[:D, LAT:] = np.asarray(fc1_w, f)
    b1v = np.concatenate([np.asarray(priv_b[task_id], f),
                          np.asarray(fc1_b, f)])
    w2 = np.zeros((_pad128(HID), HID), f)
    w2[:HID] = np.asarray(fc2_w, f)
    w3 = np.zeros((_pad128(HID), LAT), f)
    w3[:HID] = np.asarray(fc3_w, f)
    b2v = np.asarray(fc2_b, f)
    b3v = np.asarray(fc3_b, f)

    wh1 = np.zeros((2 * LAT, HT), f)
    bh1v = np.zeros(HT, f)
    wh2 = np.zeros((3 * 128, 128), f)
    bh2v = np.zeros(HT, f)
    wh3 = np.zeros((_pad128(MSK), NCLS), f)
    for t in range(T):
        c = HP * t
        wh1[:, c:c + 28] = np.asarray(h1_w[t], f)
        bh1v[c:c + 28] = np.asarray(h1_b[t], f)
        blk, off = divmod(c, 128)
        wh2[128 * blk + off:128 * blk + off + 28, off:off + 28] = \
            np.asarray(h2_w[t], f)
        bh2v[c:c + 28] = np.asarray(h2_b[t], f)
        wh3[c:c + 28, :] = np.asarray(h3_w[t], f)
        wh3[HT + t, :] = np.asarray(h3_b[t], f)

    def col_bias(parts):
        out = np.zeros((128, NBC), f)
        col = 0
        for v, msizes in parts:
            r0 = 0
            for mp_ in msizes:
                out[:mp_, col] = v[r0:r0 + mp_]
                r0 += mp_
                col += 1
        return out

    bias = col_bias([(b1v, L1_M), (b2v, L2_M), (b3v, L3_M),
                     (bh1v, H1_M), (bh2v, H1_M)])

    shared = {"w1": w1, "w2": w2, "w3": w3, "wh1": wh1, "wh2": wh2,
              "wh3": wh3, "bias": bias}

    in_maps = []
    for c in range(NCORES):
        sl = slice(c * R, (c + 1) * R)
        xT = np.zeros((_pad128(D), R), f)
        xT[:D] = x2d[sl].T
        ttc = tt[sl]
        m10 = (np.arange(T)[:, None] == ttc[None, :])
        mask = np.zeros((_pad128(MSK), R), f)
        mask[:HT] = np.repeat(m10, HP, axis=0)
        mask[HT:MSK] = m10
        m = dict(shared)
        m["xT"] = xT
        m["mask"] = mask
        in_maps.append(m)
    return in_maps


def run(inputs, trace=False, **kw):
    if "nc" not in _cache:
        _cache["nc"] = _build_program()
    nc = _cache["nc"]
    inputs = {k: v for k, v in inputs.items() if k != "x_p"}
    in_maps = _prepare_inputs(**inputs)
    res = run_bass_kernel_spmd(nc, in_maps, list(range(NCORES)),
                               trace=trace, **kw)
    outs = [res.results[c]["out"] for c in range(NCORES)]        # [10, R] each
    full = np.concatenate(outs, axis=1)                          # [10, B]
    return np.ascontiguousarray(full.T), res                     # [B, 10]


def kernel(**inputs):
    out, _ = run(inputs, trace=False)
    return out


# revision 7
# speedup vs baseline: 1.1005x; 1.0305x over previous
"""Trainium2 Bass kernel for nn_Net_89094801588965 (moe_routing).

Data-parallel over batch on 8 NeuronCores. Per-core layout puts features on
SBUF partitions and batch on the free dim, so every layer's output is directly
the next layer's moving operand (no transposes on device).

Math (identical to the reference):
  h  = relu(x @ fc1_w + b) -> relu(@fc2_w+b) -> relu(@fc3_w+b)   [B,256]
  p  = relu(x @ priv_w[task_id] + priv_b[task_id])               [B,256]
  xc = [p, h]                                                    [B,512]
  per-task heads t=0..9: a3[t] = (relu(relu(xc@h1w[t]+b)@h2w[t]+b))@h3w[t]+b
  out[b] = a3[tt[b]][b]

Device-side restructuring:
  - fc1 and the private layer share the input x -> fused into one [784,656]
    matmul (cols 0..255 = private, 256..655 = fc1).
  - head layer 1: all tasks packed as [512, 320] (task t at cols 32t..32t+27,
    zero padded) -> [320, N] activations.
  - head layer 2: block-diagonal [320, 320], 128-aligned diagonal blocks ->
    3 matmuls (tasks 0-3, 4-7, 8-9).
  - head layer 3 + routing: multiply a2 by the per-task one-hot mask (built on
    the host from tt), then contract with the stacked [330, 10] weight whose
    rows 320..329 are h3_b paired with raw one-hot mask rows -- a single
    contraction yields the routed, biased logits.  Masking only at the end is
    exact because relu/bias garbage of non-selected tasks is zeroed there.
  - all big tensors are host-padded to 128-row multiples and loaded with ONE
    dma_start each (descriptor generation is serialized at ~0.6us per DMA).
  - per-chunk head-2/head-3 work is software-pipelined into the next chunk's
    L1/L2/L3 shadow so the PE never waits on ACT/DVE round-trips.
All matmuls run as float32r (full PE rate, ~1e-4 relative error).
"""

import sys

sys.path.insert(0, "/opt/trn_rl_repo")

import numpy as np

import concourse.bass as bass
import concourse.mybir as mybir
import concourse.tile as tile
from concourse import bacc
from concourse.bass_utils import run_bass_kernel_spmd

F32 = mybir.dt.float32
F32R = mybir.dt.float32r
RELU = mybir.ActivationFunctionType.Relu

B = 65536
D = 784
HID = 400
LAT = 256
T = 10
NCLS = 10
NCORES = 8
R = B // NCORES          # rows per core
CH = 512                 # batch columns per chunk
NCH = R // CH            # chunks per core

M1 = LAT + HID           # 656 fused L1 output (private | fc1)
HP = 32                  # per-task padded head width
HT = T * HP              # 320
MSK = HT + T             # 330 mask rows: [expanded one-hot ; raw one-hot]

_cache = {}


def _ceil_tiles(n):
    full, rem = divmod(n, 128)
    return [128] * full + ([rem] if rem else [])


L1_K = _ceil_tiles(D)            # [128]*6 + [16]
L1_M = _ceil_tiles(M1)           # [128]*5 + [16]
L2_K = _ceil_tiles(HID)          # [128]*3 + [16]
L2_M = _ceil_tiles(HID)
L3_M = _ceil_tiles(LAT)          # [128, 128]
H1_K = _ceil_tiles(2 * LAT)      # [128]*4
H1_M = _ceil_tiles(HT)           # [128, 128, 64]
H3_K = _ceil_tiles(MSK)          # [128, 128, 74]

# bias column layout inside the single [128, 18] bias tensor
BC_L1, BC_L2, BC_L3, BC_H1, BC_H2 = 0, 6, 10, 12, 15
NBC = 18


def _pad128(n):
    return 128 * ((n + 127) // 128)


def _build_program():
    nc = bacc.Bacc("TRN2", target_bir_lowering=False, debug=False,
                   num_devices=NCORES)

    xT_d = nc.dram_tensor("xT", [_pad128(D), R], F32R, kind="ExternalInput")
    mk_d = nc.dram_tensor("mask", [_pad128(MSK), R], F32, kind="ExternalInput")
    w1_d = nc.dram_tensor("w1", [_pad128(D), M1], F32R, kind="ExternalInput")
    w2_d = nc.dram_tensor("w2", [_pad128(HID), HID], F32R, kind="ExternalInput")
    w3_d = nc.dram_tensor("w3", [_pad128(HID), LAT], F32R, kind="ExternalInput")
    wh1_d = nc.dram_tensor("wh1", [2 * LAT, HT], F32R, kind="ExternalInput")
    wh2_d = nc.dram_tensor("wh2", [3 * 128, 128], F32R, kind="ExternalInput")
    wh3_d = nc.dram_tensor("wh3", [_pad128(MSK), NCLS], F32R,
                           kind="ExternalInput")
    bias_d = nc.dram_tensor("bias", [128, NBC], F32, kind="ExternalInput")
    out_d = nc.dram_tensor("out", [NCLS, R], F32, kind="ExternalOutput")

    def as3d(dram):
        return dram[:].rearrange("(j p) m -> p j m", p=128)

    with tile.TileContext(nc) as tc:
        with (
            tc.tile_pool(name="wp", bufs=1) as wp,
            tc.tile_pool(name="xp", bufs=2) as xp,
            tc.tile_pool(name="mp", bufs=3) as mpool,
            tc.tile_pool(name="ap", bufs=2) as ap,
            tc.tile_pool(name="op", bufs=2) as op,
            tc.tile_pool(name="ps", bufs=8, space="PSUM") as ps,
        ):
            # ---- resident weights: one DMA per tensor --------------------
            def load_w3d(dram, nk, ncols, tag):
                t = wp.tile([128, nk, ncols], F32R, tag=tag)
                nc.sync.dma_start(t[:], as3d(dram))
                return t

            def load_x_chunk(ci):
                t = xp.tile([128, len(L1_K), CH], F32R, tag="x")
                nc.sync.dma_start(
                    t[:], as3d(xT_d)[:, :, ci * CH:(ci + 1) * CH])
                return t

            # k-tile-granular interleaved load of W1 and chunk-0 x so the
            # first matmul can start after ~0.6 MB instead of ~4 MB
            w1 = wp.tile([128, len(L1_K), M1], F32R, tag="w1")
            x0 = xp.tile([128, len(L1_K), CH], F32R, tag="x")
            w1_3d, x_3d = as3d(w1_d), as3d(xT_d)
            for ki in range(len(L1_K)):
                nc.sync.dma_start(w1[:, ki, :], w1_3d[:, ki, :])
                nc.sync.dma_start(x0[:, ki, :], x_3d[:, ki, 0:CH])
            x1 = load_x_chunk(1)
            w2 = load_w3d(w2_d, len(L2_K), HID, "w2")
            w3 = load_w3d(w3_d, len(L2_K), LAT, "w3")
            wh1 = load_w3d(wh1_d, len(H1_K), HT, "wh1")
            wh2 = load_w3d(wh2_d, 3, 128, "wh2")
            wh3 = load_w3d(wh3_d, len(H3_K), NCLS, "wh3")
            bias = wp.tile([128, NBC], F32, tag="bias")
            nc.sync.dma_start(bias[:], bias_d[:])

            # ---- helpers -------------------------------------------------
            def mm_layer(rhs3, ksizes, w3t, msizes, rhs_list=None):
                """K-accumulated matmuls; rhs3 is a [128, nk, CH] tile or
                rhs_list a list of [kp, CH] tiles."""
                psums = []
                c0 = 0
                for mi, mp_ in enumerate(msizes):
                    pt = ps.tile([mp_, CH], F32, tag="ps")
                    nk = len(ksizes)
                    for ki, kp in enumerate(ksizes):
                        rhs = (rhs3[0:kp, ki, :] if rhs3 is not None
                               else rhs_list[ki][:])
                        nc.tensor.matmul(
                            pt[:], w3t[0:kp, ki, c0:c0 + mp_], rhs,
                            start=(ki == 0), stop=(ki == nk - 1),
                        )
                    psums.append(pt)
                    c0 += mp_
                return psums

            def act_relu(psums, bcol, msizes, tag, eng="act"):
                outs = []
                for mi, mp_ in enumerate(msizes):
                    t = ap.tile([mp_, CH], F32R, tag=f"{tag}{mi}")
                    bap = bias[:mp_, bcol + mi:bcol + mi + 1]
                    if eng == "act":
                        nc.scalar.activation(t[:], psums[mi][:], RELU,
                                             bias=bap, scale=1.0)
                    else:
                        nc.vector.tensor_scalar(
                            t[:], psums[mi][:], bap, 0.0,
                            op0=mybir.AluOpType.add, op1=mybir.AluOpType.max)
                    outs.append(t)
                return outs

            tails_h2 = []
            tails_h3 = []

            for ci in range(NCH):
                cs = ci * CH
                xk = x0 if ci == 0 else (x1 if ci == 1 else load_x_chunk(ci))
                mk = mpool.tile([128, len(H3_K), CH], F32, tag="mask")
                nc.sync.dma_start(mk[:], as3d(mk_d)[:, :, cs:cs + CH])

                # L1 fused (private | fc1)
                ps1 = mm_layer(xk, L1_K, w1, L1_M)
                a_l1 = act_relu(ps1, BC_L1, L1_M, "l1o")
                x2 = [a_l1[0], a_l1[1]]
                h1t = [a_l1[2], a_l1[3], a_l1[4], a_l1[5]]

                # previous chunk's head-2 runs inside this chunk's L1 shadow
                while tails_h2:
                    tails_h2.pop(0)()

                ps2 = mm_layer(None, L2_K, w2, L2_M, rhs_list=h1t)
                h2t = act_relu(ps2, BC_L2, L2_M, "l2o", eng="dve")
                ps3 = mm_layer(None, L2_K, w3, L3_M, rhs_list=h2t)
                x2 += act_relu(ps3, BC_L3, L3_M, "l3o", eng="dve")

                # previous chunk's head-3 + store
                while tails_h3:
                    tails_h3.pop(0)()

                ph1 = mm_layer(None, H1_K, wh1, H1_M, rhs_list=x2)
                a1 = act_relu(ph1, BC_H1, H1_M, "a1")

                def tail_h2(a1=a1, mk=mk):
                    ph2 = []
                    for i, kp in enumerate(H1_M):
                        pt = ps.tile([kp, CH], F32, tag="ps")
                        nc.tensor.matmul(pt[:], wh2[0:kp, i, 0:kp], a1[i][:],
                                         start=True, stop=True)
                        ph2.append(pt)
                    a2 = act_relu(ph2, BC_H2, H1_M, "a2")
                    # mask the per-task activations; the 74-row tile also
                    # carries the raw one-hot rows that select h3_b
                    am = []
                    for i in range(2):
                        t = ap.tile([128, CH], F32R, tag=f"am{i}")
                        nc.vector.tensor_tensor(t[:], a2[i][:], mk[:, i, :],
                                                op=mybir.AluOpType.mult)
                        am.append(t)
                    t2 = ap.tile([H3_K[2], CH], F32R, tag="am2")
                    nc.vector.tensor_tensor(t2[0:64, :], a2[2][:],
                                            mk[0:64, 2, :],
                                            op=mybir.AluOpType.mult)
                    nc.vector.tensor_copy(t2[64:, :], mk[64:H3_K[2], 2, :])
                    am.append(t2)
                    return am

                def tail_h3(am, cs=cs):
                    po = ps.tile([NCLS, CH], F32, tag="ps")
                    for i, kp in enumerate(H3_K):
                        nc.tensor.matmul(po[:], wh3[0:kp, i, :], am[i][:],
                                         start=(i == 0), stop=(i == 2))
                    ot = op.tile([NCLS, CH], F32, tag="o")
                    nc.scalar.copy(ot[:], po[:])
                    nc.sync.dma_start(out_d[:, cs:cs + CH], ot[:])

                def chain(t2=tail_h2, t3=tail_h3):
                    am = t2()
                    tails_h3.append(lambda: t3(am))

                tails_h2.append(chain)

            while tails_h2:
                tails_h2.pop(0)()
            while tails_h3:
                tails_h3.pop(0)()

    nc.compile()
    return nc


def _prepare_inputs(x_s, tt, task_id,
                    fc1_w, fc1_b, fc2_w, fc2_b, fc3_w, fc3_b,
                    priv_w, priv_b, h1_w, h1_b, h2_w, h2_b, h3_w, h3_b):
    f = np.float32
    task_id = int(task_id)

    x2d = np.asarray(x_s, f).reshape(B, D)
    tt = np.asarray(tt).astype(np.int64).reshape(B)

    w1 = np.zeros((_pad128(D), M1), f)
    w1[:D, :LAT] = np.asarray(priv_w[task_id], f)
    w1[:D, LAT:] = np.asarray(fc1_w, f)
    b1v = np.concatenate([np.asarray(priv_b[task_id], f),
                          np.asarray(fc1_b, f)])
    w2 = np.zeros((_pad128(HID), HID), f)
    w2[:HID] = np.asarray(fc2_w, f)
    w3 = np.zeros((_pad128(HID), LAT), f)
    w3[:HID] = np.asarray(fc3_w, f)
    b2v = np.asarray(fc2_b, f)
    b3v = np.asarray(fc3_b, f)

    wh1 = np.zeros((2 * LAT, HT), f)
    bh1v = np.zeros(HT, f)
    wh2 = np.zeros((3 * 128, 128), f)
    bh2v = np.zeros(HT, f)
    wh3 = np.zeros((_pad128(MSK), NCLS), f)
    for t in range(T):
        c = HP * t
        wh1[:, c:c + 28] = np.asarray(h1_w[t], f)
        bh1v[c:c + 28] = np.asarray(h1_b[t], f)
        blk, off = divmod(c, 128)
        wh2[128 * blk + off:128 * blk + off + 28, off:off + 28] = \
            np.asarray(h2_w[t], f)
        bh2v[c:c + 28] = np.asarray(h2_b[t], f)
        wh3[c:c + 28, :] = np.asarray(h3_w[t], f)
        wh3[HT + t, :] = np.asarray(h3_b[t], f)

    def col_bias(parts):
        out = np.zeros((128, NBC), f)
        col = 0
        for v, msizes in parts:
            r0 = 0
            for mp_ in msizes:
                out[:mp_, col] = v[r0:r0 + mp_]
                r0 += mp_
                col += 1
        return out

    bias = col_bias([(b1v, L1_M), (b2v, L2_M), (b3v, L3_M),
                     (bh1v, H1_M), (bh2v, H1_M)])

    shared = {"w1": w1, "w2": w2, "w3": w3, "wh1": wh1, "wh2": wh2,
              "wh3": wh3, "bias": bias}

    in_maps = []
    for c in range(NCORES):
        sl = slice(c * R, (c + 1) * R)
        xT = np.zeros((_pad128(D), R), f)
        xT[:D] = x2d[sl].T
        ttc = tt[sl]
        m10 = (np.arange(T)[:, None] == ttc[None, :])
        mask = np.zeros((_pad128(MSK), R), f)
        mask[:HT] = np.repeat(m10, HP, axis=0)
        mask[HT:MSK] = m10
        m = dict(shared)
        m["xT"] = xT
        m["mask"] = mask
        in_maps.append(m)
    return in_maps


def run(inputs, trace=False, **kw):
    if "nc" not in _cache:
        _cache["nc"] = _build_program()
    nc = _cache["nc"]
    inputs = {k: v for k, v in inputs.items() if k != "x_p"}
    in_maps = _prepare_inputs(**inputs)
    res = run_bass_kernel_spmd(nc, in_maps, list(range(NCORES)),
                               trace=trace, **kw)
    outs = [res.results[c]["out"] for c in range(NCORES)]        # [10, R] each
    full = np.concatenate(outs, axis=1)                          # [10, B]
    return np.ascontiguousarray(full.T), res                     # [B, 10]


def kernel(**inputs):
    out, _ = run(inputs, trace=False)
    return out
